# revision 17
# baseline (speedup 1.0000x reference)
"""NeighborhoodAttention2D Trainium2 kernel (8-core data parallel over batch).

Math (matches reference.py):
  dot(h,p)  = sum_{c in head h} s*(q_c(p)+bq_c)*(k_c(p)+bk_c),  s = hd^-0.5
  logit_ij(p) = dot(p + (i,j)) + rpb[h,i,j]      (circular shifts)
  attn      = softmax over the 49 (i,j)
  out_attn(c,p) = sum_ij attn_ij(p) * (v_c + bv_c)(p + (i,j))
Max-free softmax: E = exp(dot); R = exp(rpb);
  num = sum_ij R_ij*(E*v)_shift;  Z = sum_ij R_ij*E_shift;  out = num/Z + bv
(bv folded into the proj bias: proj_w @ bv + proj_b.)

Layout strategy: the 49-tap conv runs in a spatially-transposed layout
(partitions = x) as banded-circulant bf16 matmuls on TensorE:
  num_T[xd, y, c] = sum_i sum_xs band[h,i][xs,xd] * U_T[xs, y+i, c]
where band[h,i][xs,xd] = R[h,i,(xs-xd) mod 112] masks a 7-wide circulant.
U_T = E*(v+bv) is produced directly in transposed form by using the x tile
as the matmul lhsT (out partitions = positions). The x-wrap is inside the
band matrices; the y-wrap uses 6 halo rows. Z rides along as a 33rd
channel per head and the num/Z division happens on the Pool engine with a
stride-0-broadcast psum divisor. All weight preprocessing (transposes,
scale folding, exp(rpb), band construction, bf16 casts) happens on the
host in numpy.
"""
import sys
import contextlib
import numpy as np

sys.path.insert(0, '/opt/trn_rl_repo')

import concourse.bass as bass
import concourse.bacc as bacc
import concourse.mybir as mybir
from concourse import tile
from concourse.bass_utils import run_bass_kernel_spmd
from ml_dtypes import bfloat16

# ---- problem constants ----
B, C, H, W = 8, 128, 112, 112
NH, HD, KS = 4, 32, 7
HW = H * W                        # 12544
YP = H + KS - 1                   # 118 (y-padded)
SCALE = HD ** (-0.5)
NG = H // 4                       # 28 4-row groups
TN = 4 * W                        # 448 positions per group
NCH = 8                           # tap chunks (14 rows each)
CHR = H // NCH                    # 14 rows per chunk

F32 = mybir.dt.float32
BF16 = mybir.dt.bfloat16
AL = mybir.AluOpType
AF = mybir.ActivationFunctionType

# tap chunks (y0, rows): small early chunks start tap work sooner
CHUNKS = [(14 * i, 14) for i in range(8)]

def _chunk_of_row(r):
    for ci, (y0, rows) in enumerate(CHUNKS):
        if y0 <= r < y0 + rows:
            return ci
    raise AssertionError

# iteration at which chunk ci's tap units may be emitted: the taps read
# input rows y0..y0+rows+5, which exist once a_back(g-1) covered them
GATES = {}
for ci, (y0, rows) in enumerate(CHUNKS):
    need = y0 + rows + 5            # top input row (halo-free chunks)
    g = -(-(need + 1) // 4)         # a_back(g-1) covers rows <= 4g-1
    if need >= H:
        g = NG + 1                  # post-loop (needs halo)
    GATES.setdefault(min(g, NG + 1), []).append(ci)
AFTER_B = {ci: [t for t in range(NG) if _chunk_of_row(4 * t + 3) == ci]
           for ci in range(len(CHUNKS))}


def build_nc(zero_qkv_bias=True):
    nc = bacc.Bacc(target_bir_lowering=False)

    x_d = nc.dram_tensor("x", [C, HW], BF16, kind="ExternalInput")
    wq_d = nc.dram_tensor("wq", [C, C], BF16, kind="ExternalInput")
    wk_d = nc.dram_tensor("wk", [C, C], BF16, kind="ExternalInput")
    wvT_d = nc.dram_tensor("wvT", [C, C], BF16, kind="ExternalInput")
    hm4_d = nc.dram_tensor("hm4", [C, NH], BF16, kind="ExternalInput")
    band_d = nc.dram_tensor("band", [W, NH * KS * W], BF16, kind="ExternalInput")
    projw_d = nc.dram_tensor("projw", [C, C], BF16, kind="ExternalInput")
    ident_d = nc.dram_tensor("ident", [W, W], BF16, kind="ExternalInput")
    pbias_d = nc.dram_tensor("pbias", [C, 1], F32, kind="ExternalInput")
    if not zero_qkv_bias:
        bq_d = nc.dram_tensor("bq", [C, 1], F32, kind="ExternalInput")
        bk_d = nc.dram_tensor("bk", [C, 1], F32, kind="ExternalInput")
    out_d = nc.dram_tensor("out", [C, HW], BF16, kind="ExternalOutput")

    with tile.TileContext(nc) as tc, contextlib.ExitStack() as ctx:
        sing = ctx.enter_context(tc.tile_pool(name="sing", bufs=1))
        work = ctx.enter_context(tc.tile_pool(name="work", bufs=2))
        outp = ctx.enter_context(tc.tile_pool(name="outp", bufs=2))
        psum = ctx.enter_context(tc.tile_pool(name="psum", bufs=1, space="PSUM"))

        # ---- PSUM: 8 banks, statically placed, subtile deps ----
        qp = psum.tile([C, TN], F32, tag="qp")
        kp = psum.tile([C, TN], F32, tag="kp")
        vd = [psum.tile([W, 4, C], F32, tag=f"vd{i}", name=f"vd{i}")
              for i in range(2)]
        npt = [psum.tile([W, CHR, HD + 1], F32, tag=f"np{i}", name=f"np{i}")
               for i in range(2)]
        tp = psum.tile([C, 8, W], BF16, tag="tp")
        pod = psum.tile([C, TN + 8 * NH], F32, tag="pod")  # proj | 2x dot regions

        # ---- weights (scalar queue) and x (sync queue), first-needed first ----
        wq_t = sing.tile([C, C], BF16, tag="wq")
        wk_t = sing.tile([C, C], BF16, tag="wk")
        wvT_t = sing.tile([C, C], BF16, tag="wvT")
        hm4_t = sing.tile([C, NH], BF16, tag="hm4")
        band_t = sing.tile([W, NH * KS, W], BF16, tag="band")
        projw_t = sing.tile([C, C], BF16, tag="projw")
        ident_t = sing.tile([W, W], BF16, tag="ident")
        pbias_t = sing.tile([C, 1], F32, tag="pbias")
        x_bf = sing.tile([C, HW], BF16, tag="x_bf")

        nc.sync.dma_start(out=wq_t, in_=wq_d.ap())
        nc.sync.dma_start(out=wk_t, in_=wk_d.ap())
        # x chunks: rows [0:8), [8:16), then 16-row chunks
        xrow = [0, 4, 8, 16, 32, 48, 64, 80, 96, 112]
        for d in range(len(xrow) - 1):
            r0, r1 = xrow[d], xrow[d + 1]
            nc.scalar.dma_start(out=x_bf[:, r0 * W:r1 * W],
                                in_=x_d.ap()[:, r0 * W:r1 * W])
        nc.sync.dma_start(out=wvT_t, in_=wvT_d.ap())
        nc.sync.dma_start(out=hm4_t, in_=hm4_d.ap())
        if not zero_qkv_bias:
            bq_t = sing.tile([C, 1], F32, tag="bq")
            bk_t = sing.tile([C, 1], F32, tag="bk")
            nc.sync.dma_start(out=bq_t, in_=bq_d.ap())
            nc.sync.dma_start(out=bk_t, in_=bk_d.ap())
        nc.sync.dma_start(out=band_t[:, :, :].rearrange("p a b -> p (a b)"),
                            in_=band_d.ap())
        nc.sync.dma_start(out=projw_t, in_=projw_d.ap())
        nc.sync.dma_start(out=ident_t, in_=ident_d.ap())
        nc.sync.dma_start(out=pbias_t, in_=pbias_d.ap())

        U_T = sing.tile([W, YP, NH, HD + 1], BF16, tag="U_T")
        E_sb = sing.tile([W, H, NH], F32, tag="E_sb")
        attn_sb = sing.tile([W, H, C], BF16, tag="attn_sb")

        dotv = [pod[0:W, TN + 16 * i:TN + 16 * (i + 1)]
            .rearrange("p (a b) -> p a b", a=4) for i in range(2)]

        # ---------- phase A (per 4-row group) ----------
        def a_front(g):
            y0 = 4 * g
            nc.tensor.matmul(qp[:], wq_t[:], x_bf[:, y0 * W:(y0 + 4) * W],
                             start=True, stop=True)
            nc.tensor.matmul(kp[:], wk_t[:], x_bf[:, y0 * W:(y0 + 4) * W],
                             start=True, stop=True)
            v = vd[g % 2]
            for r in range(4):
                nc.tensor.matmul(v[:, r, :],
                                 x_bf[:, (y0 + r) * W:(y0 + r + 1) * W],
                                 wvT_t[:], start=True, stop=True)
            qk_bf = work.tile([C, TN], BF16, tag="qk_bf")
            if zero_qkv_bias:
                nc.vector.tensor_tensor(qk_bf[:], qp[:], kp[:], AL.mult)
            else:
                k_sb = work.tile([C, TN], BF16, tag="k_sb")
                nc.scalar.activation(k_sb[:], kp[:], AF.Identity,
                                     bias=bk_t[:, 0:1])
                nc.vector.scalar_tensor_tensor(qk_bf[:], qp[:], bq_t[:, 0:1],
                                               k_sb[:], AL.add, AL.mult)
            return qk_bf

        def a_back(g, qk_bf):
            y0 = 4 * g
            dv = dotv[g % 2]
            for r in range(4):
                nc.tensor.matmul(dv[:, r, :],
                                 qk_bf[:, r * W:(r + 1) * W], hm4_t[:],
                                 start=True, stop=True)
            nc.scalar.activation(E_sb[:, y0:y0 + 4, :], dv[:, :, :], AF.Exp)
            v = vd[g % 2]
            esl = E_sb[:, y0:y0 + 4, :]
            e_bc = bass.AP(tensor=esl.tensor, offset=esl.offset,
                           ap=[list(esl.ap[0]), [NH, 4], [1, NH], [0, HD]])
            nc.vector.tensor_tensor(
                U_T[:, y0:y0 + 4, :, 0:HD],
                v[:, :, :].rearrange("p y (h g) -> p y h g", h=NH),
                e_bc, AL.mult)
            nc.vector.tensor_copy(U_T[:, y0:y0 + 4, :, HD], esl)

        # ---------- phase B: taps + divide (spread (chunk, head) units) ----------
        # C-groups of a completed chunk are interleaved between units so the
        # phase-C work spreads evenly across the tap work
        pending_c = []
        unit_q = []
        unit_n = 0

        def tap_unit(ch, h):
            nonlocal unit_n
            y0, rows = CHUNKS[ch]
            np_ps = npt[unit_n % 2]
            unit_n += 1
            for i in range(KS):
                nc.tensor.matmul(np_ps[:, 0:rows, :],
                                 band_t[:, h * KS + i, :],
                                 U_T[:, y0 + i:y0 + i + rows, h, :],
                                 start=(i == 0), stop=(i == KS - 1))
            mp = np_ps[:, :, :]
            nh_half = (rows + 7) // 8   # 1 divide for 7 rows, 2 for 14
            step = rows // nh_half
            for half in range(nh_half):
                ys = half * step
                zbc = bass.AP(tensor=mp.tensor,
                              offset=mp.offset + ys * (HD + 1) + HD,
                              ap=[list(mp.ap[0]), [HD + 1, step], [0, HD]])
                nc.gpsimd.tensor_tensor(
                    attn_sb[:, y0 + ys:y0 + ys + step,
                            h * HD:(h + 1) * HD],
                    np_ps[:, ys:ys + step, 0:HD], zbc, AL.divide)
            if h == NH - 1:
                pending_c.extend(AFTER_B[ch])

        def pop_units(k):
            while k > 0 and unit_q:
                ch, h = unit_q.pop(0)
                tap_unit(ch, h)
                if pending_c:
                    run_c(pending_c.pop(0))
                k -= 1

        # ---------- phase C: transpose -> proj -> out (per 4-row group) ----------
        c_state = {"t": None, "nat": None, "out_sb": None}

        def c_front(t):
            y0 = 4 * t
            s0 = (t % 2) * 4
            for r in range(4):
                nc.tensor.transpose(tp[:, s0 + r, :], attn_sb[:, y0 + r, :],
                                    ident_t[:])
            attn_nat = work.tile([C, TN], BF16, tag="attn_nat")
            nc.scalar.activation(attn_nat[:, :].rearrange("p (a b) -> p a b", a=4),
                                 tp[:, s0:s0 + 4, :], AF.Copy)
            return attn_nat

        def c_back(t, attn_nat, out_sb):
            nc.tensor.matmul(pod[:, 0:TN], projw_t[:], attn_nat[:],
                             start=True, stop=True)
            s = (t % 4) * TN
            nc.vector.tensor_scalar(out_sb[:, s:s + TN], pod[:, 0:TN],
                                    pbias_t[:, 0:1], None, AL.add)
            if t // 4 == 6:     # last quarter: store per group (short tail)
                nc.sync.dma_start(
                    out=out_d.ap()[:, 4 * t * W:4 * (t + 1) * W],
                    in_=out_sb[:, s:s + TN])
            elif t % 4 == 3:
                q = t // 4
                nc.sync.dma_start(
                    out=out_d.ap()[:, q * 4 * TN:(q + 1) * 4 * TN],
                    in_=out_sb)

        def run_c(t):
            # pipelined: emit proj/out for the previous group, then
            # transposes for this one
            if c_state["t"] is not None:
                c_back(c_state["t"], c_state["nat"], c_state["out_sb"])
            if t is None:
                c_state["t"] = None
                return
            if t % 4 == 0:
                c_state["out_sb"] = outp.tile([C, 4 * TN], BF16, tag="out_sb",
                                              name="out_sb")
            c_state["nat"] = c_front(t)
            c_state["t"] = t

        # ---------- emission schedule (software-pipelined, lag 1) ----------
        qk_prev = None
        for g in range(NG):
            qk_cur = a_front(g)
            if g >= 1:
                a_back(g - 1, qk_prev)
            qk_prev = qk_cur
            if g == 2:
                # y halo: rows 112..117 = rows 0..5 (incl. Z channel)
                nc.vector.tensor_copy(U_T[:, H:YP, :, :], U_T[:, 0:KS - 1, :, :])
            for ch in GATES.get(g, []):
                unit_q.extend((ch, h) for h in range(NH))
            pop_units(2)
        a_back(NG - 1, qk_prev)
        for ch in GATES.get(NG + 1, []):
            unit_q.extend((ch, h) for h in range(NH))
        pop_units(len(unit_q))
        while pending_c:
            run_c(pending_c.pop(0))
        run_c(None)

    nc.compile()
    return nc


def prep_inputs(x_b, qkv_w, qkv_b, rpb, proj_w, proj_b, zero_qkv_bias=True):
    """Host-side preprocessing of one batch element + shared weights."""
    qkv_w = np.asarray(qkv_w, dtype=np.float32)
    qkv_b = np.asarray(qkv_b, dtype=np.float32)
    rpb = np.asarray(rpb, dtype=np.float32).reshape(NH, KS, KS)
    proj_w = np.asarray(proj_w, dtype=np.float32)
    proj_b = np.asarray(proj_b, dtype=np.float32)

    wq = (SCALE * qkv_w[0:C]).T.astype(bfloat16)          # [a, c_out]
    wk = qkv_w[C:2 * C].T.astype(bfloat16)
    wvT = qkv_w[2 * C:3 * C].T.astype(bfloat16)
    bv = qkv_b[2 * C:3 * C]
    hm4 = np.zeros((C, NH), np.float32)
    for h in range(NH):
        hm4[h * HD:(h + 1) * HD, h] = 1.0
    hm4 = hm4.astype(bfloat16)
    R = np.exp(rpb)                                        # [NH, KS, KS]
    xs = np.arange(W)[:, None]
    xd = np.arange(W)[None, :]
    jm = (xs - xd) % W
    mask = jm < KS
    jc = np.minimum(jm, KS - 1)
    band = np.zeros((W, NH, KS, W), np.float32)
    for h in range(NH):
        for i in range(KS):
            band[:, h, i, :] = np.where(mask, R[h, i][jc], 0.0)
    band = band.reshape(W, NH * KS * W).astype(bfloat16)
    projw = proj_w.T.astype(bfloat16)
    pbias = (proj_w @ bv + proj_b).reshape(C, 1).astype(np.float32)
    ident = np.eye(W, dtype=bfloat16)
    m = {"x": x_b.astype(bfloat16), "wq": wq, "wk": wk, "wvT": wvT,
         "hm4": hm4, "band": band, "projw": projw, "ident": ident,
         "pbias": pbias}
    if not zero_qkv_bias:
        m["bq"] = (SCALE * qkv_b[0:C]).reshape(C, 1).astype(np.float32)
        m["bk"] = qkv_b[C:2 * C].reshape(C, 1).astype(np.float32)
    return m


_NC = None
_NC_ZB = None


def kernel(x, qkv_w, qkv_b, rpb, proj_w, proj_b):
    global _NC, _NC_ZB
    qkv_b = np.asarray(qkv_b, dtype=np.float32)
    zb = bool(np.all(qkv_b[0:2 * C] == 0.0))
    if _NC is None or _NC_ZB != zb:
        _NC = build_nc(zero_qkv_bias=zb)
        _NC_ZB = zb
    x = np.ascontiguousarray(np.asarray(x, dtype=np.float32))
    shared = prep_inputs(np.zeros((C, HW), np.float32),
                         qkv_w, qkv_b, rpb, proj_w, proj_b, zero_qkv_bias=zb)
    in_maps = []
    for b in range(B):
        m = dict(shared)
        m["x"] = x[b].reshape(C, HW).astype(bfloat16)
        in_maps.append(m)
    res = run_bass_kernel_spmd(_NC, in_maps, list(range(B)), trace=False)
    return np.stack([np.asarray(res.results[b]["out"], dtype=np.float32)
                     .reshape(C, H, W) for b in range(B)])


# revision 18
# speedup vs baseline: 1.0722x; 1.0722x over previous
"""NeighborhoodAttention2D Trainium2 kernel (8-core data parallel over batch).

Math (matches reference.py):
  dot(h,p)  = sum_{c in head h} s*(q_c(p)+bq_c)*(k_c(p)+bk_c),  s = hd^-0.5
  logit_ij(p) = dot(p + (i,j)) + rpb[h,i,j]      (circular shifts)
  attn      = softmax over the 49 (i,j)
  out_attn(c,p) = sum_ij attn_ij(p) * (v_c + bv_c)(p + (i,j))
Max-free softmax: E = exp(dot); R = exp(rpb);
  num = sum_ij R_ij*(E*v)_shift;  Z = sum_ij R_ij*E_shift;  out = num/Z + bv
(bv folded into the proj bias: proj_w @ bv + proj_b.)

Layout strategy: the 49-tap conv runs in a spatially-transposed layout
(partitions = x) as banded-circulant bf16 matmuls on TensorE:
  num_T[xd, y, c] = sum_i sum_xs band[h,i][xs,xd] * U_T[xs, y+i, c]
where band[h,i][xs,xd] = R[h,i,(xs-xd) mod 112] masks a 7-wide circulant.
U_T = E*(v+bv) is produced directly in transposed form by using the x tile
as the matmul lhsT (out partitions = positions). The x-wrap is inside the
band matrices; the y-wrap uses 6 halo rows. Z rides along as a 33rd
channel per head and the num/Z division happens on the Pool engine with a
stride-0-broadcast psum divisor. All weight preprocessing (transposes,
scale folding, exp(rpb), band construction, bf16 casts) happens on the
host in numpy.
"""
import sys
import contextlib
import numpy as np

sys.path.insert(0, '/opt/trn_rl_repo')

import concourse.bass as bass
import concourse.bacc as bacc
import concourse.mybir as mybir
from concourse import tile
from concourse.bass_utils import run_bass_kernel_spmd
from ml_dtypes import bfloat16

# ---- problem constants ----
B, C, H, W = 8, 128, 112, 112
NH, HD, KS = 4, 32, 7
HW = H * W                        # 12544
YP = H + KS - 1                   # 118 (y-padded)
SCALE = HD ** (-0.5)
NG = H // 4                       # 28 4-row groups
TN = 4 * W                        # 448 positions per group
NCH = 8                           # tap chunks (14 rows each)
CHR = H // NCH                    # 14 rows per chunk

F32 = mybir.dt.float32
BF16 = mybir.dt.bfloat16
AL = mybir.AluOpType
AF = mybir.ActivationFunctionType

# tap chunks (y0, rows): small early chunks start tap work sooner
CHUNKS = [(14 * i, 14) for i in range(8)]

def _chunk_of_row(r):
    for ci, (y0, rows) in enumerate(CHUNKS):
        if y0 <= r < y0 + rows:
            return ci
    raise AssertionError

# iteration at which chunk ci's tap units may be emitted: the taps read
# input rows y0..y0+rows+5, which exist once a_back(g-1) covered them
GATES = {}
for ci, (y0, rows) in enumerate(CHUNKS):
    need = y0 + rows + 5            # top input row (halo-free chunks)
    g = -(-(need + 1) // 4) + 1     # one extra iteration of slack
    if need >= H:
        g = NG + 1                  # post-loop (needs halo)
    GATES.setdefault(min(g, NG + 1), []).append(ci)
AFTER_B = {ci: [t for t in range(NG) if _chunk_of_row(4 * t + 3) == ci]
           for ci in range(len(CHUNKS))}


def build_nc(zero_qkv_bias=True):
    nc = bacc.Bacc(target_bir_lowering=False)

    x_d = nc.dram_tensor("x", [C, HW], BF16, kind="ExternalInput")
    wq_d = nc.dram_tensor("wq", [C, C], BF16, kind="ExternalInput")
    wk_d = nc.dram_tensor("wk", [C, C], BF16, kind="ExternalInput")
    wvT_d = nc.dram_tensor("wvT", [C, C], BF16, kind="ExternalInput")
    hm4_d = nc.dram_tensor("hm4", [C, NH], BF16, kind="ExternalInput")
    band_d = nc.dram_tensor("band", [W, NH * KS * W], BF16, kind="ExternalInput")
    projw_d = nc.dram_tensor("projw", [C, C], BF16, kind="ExternalInput")
    ident_d = nc.dram_tensor("ident", [W, W], BF16, kind="ExternalInput")
    pbias_d = nc.dram_tensor("pbias", [C, 1], F32, kind="ExternalInput")
    if not zero_qkv_bias:
        bq_d = nc.dram_tensor("bq", [C, 1], F32, kind="ExternalInput")
        bk_d = nc.dram_tensor("bk", [C, 1], F32, kind="ExternalInput")
    out_d = nc.dram_tensor("out", [C, HW], BF16, kind="ExternalOutput")

    with tile.TileContext(nc) as tc, contextlib.ExitStack() as ctx:
        sing = ctx.enter_context(tc.tile_pool(name="sing", bufs=1))
        work = ctx.enter_context(tc.tile_pool(name="work", bufs=2))
        outp = ctx.enter_context(tc.tile_pool(name="outp", bufs=2))
        psum = ctx.enter_context(tc.tile_pool(name="psum", bufs=1, space="PSUM"))

        # ---- PSUM: 8 banks, statically placed, subtile deps ----
        qp = psum.tile([C, TN], F32, tag="qp")
        kp = psum.tile([C, TN], F32, tag="kp")
        vd = [psum.tile([W, 4, C], F32, tag=f"vd{i}", name=f"vd{i}")
              for i in range(2)]
        npt = [psum.tile([W, CHR, HD + 1], F32, tag=f"np{i}", name=f"np{i}")
               for i in range(2)]
        tp = psum.tile([C, 8, W], BF16, tag="tp")
        pod = psum.tile([C, TN + 8 * NH], F32, tag="pod")  # proj | 2x dot regions

        # ---- weights (scalar queue) and x (sync queue), first-needed first ----
        wq_t = sing.tile([C, C], BF16, tag="wq")
        wk_t = sing.tile([C, C], BF16, tag="wk")
        wvT_t = sing.tile([C, C], BF16, tag="wvT")
        hm4_t = sing.tile([C, NH], BF16, tag="hm4")
        band_t = sing.tile([W, NH * KS, W], BF16, tag="band")
        projw_t = sing.tile([C, C], BF16, tag="projw")
        ident_t = sing.tile([W, W], BF16, tag="ident")
        pbias_t = sing.tile([C, 1], F32, tag="pbias")
        x_bf = sing.tile([C, HW], BF16, tag="x_bf")

        nc.sync.dma_start(out=wq_t, in_=wq_d.ap())
        nc.sync.dma_start(out=wk_t, in_=wk_d.ap())
        # x chunks: rows [0:8), [8:16), then 16-row chunks
        xrow = [0, 4, 8, 16, 32, 48, 64, 80, 96, 112]
        for d in range(len(xrow) - 1):
            r0, r1 = xrow[d], xrow[d + 1]
            nc.scalar.dma_start(out=x_bf[:, r0 * W:r1 * W],
                                in_=x_d.ap()[:, r0 * W:r1 * W])
        nc.sync.dma_start(out=wvT_t, in_=wvT_d.ap())
        nc.sync.dma_start(out=hm4_t, in_=hm4_d.ap())
        if not zero_qkv_bias:
            bq_t = sing.tile([C, 1], F32, tag="bq")
            bk_t = sing.tile([C, 1], F32, tag="bk")
            nc.sync.dma_start(out=bq_t, in_=bq_d.ap())
            nc.sync.dma_start(out=bk_t, in_=bk_d.ap())
        nc.sync.dma_start(out=band_t[:, :, :].rearrange("p a b -> p (a b)"),
                            in_=band_d.ap())
        nc.sync.dma_start(out=projw_t, in_=projw_d.ap())
        nc.sync.dma_start(out=ident_t, in_=ident_d.ap())
        nc.sync.dma_start(out=pbias_t, in_=pbias_d.ap())

        U_T = sing.tile([W, YP, NH, HD + 1], BF16, tag="U_T")
        E_sb = sing.tile([W, H, NH], F32, tag="E_sb")
        attn_sb = sing.tile([W, H, C], BF16, tag="attn_sb")

        dotv = [pod[0:W, TN + 16 * i:TN + 16 * (i + 1)]
            .rearrange("p (a b) -> p a b", a=4) for i in range(2)]

        # ---------- phase A (per 4-row group) ----------
        def a_front(g):
            y0 = 4 * g
            nc.tensor.matmul(qp[:], wq_t[:], x_bf[:, y0 * W:(y0 + 4) * W],
                             start=True, stop=True)
            nc.tensor.matmul(kp[:], wk_t[:], x_bf[:, y0 * W:(y0 + 4) * W],
                             start=True, stop=True)
            v = vd[g % 2]
            for r in range(4):
                nc.tensor.matmul(v[:, r, :],
                                 x_bf[:, (y0 + r) * W:(y0 + r + 1) * W],
                                 wvT_t[:], start=True, stop=True)
            qk_bf = work.tile([C, TN], BF16, tag="qk_bf")
            if zero_qkv_bias:
                nc.vector.tensor_tensor(qk_bf[:], qp[:], kp[:], AL.mult)
            else:
                k_sb = work.tile([C, TN], BF16, tag="k_sb")
                nc.scalar.activation(k_sb[:], kp[:], AF.Identity,
                                     bias=bk_t[:, 0:1])
                nc.vector.scalar_tensor_tensor(qk_bf[:], qp[:], bq_t[:, 0:1],
                                               k_sb[:], AL.add, AL.mult)
            return qk_bf

        def a_back(g, qk_bf):
            y0 = 4 * g
            dv = dotv[g % 2]
            for r in range(4):
                nc.tensor.matmul(dv[:, r, :],
                                 qk_bf[:, r * W:(r + 1) * W], hm4_t[:],
                                 start=True, stop=True)
            nc.scalar.activation(E_sb[:, y0:y0 + 4, :], dv[:, :, :], AF.Exp)
            v = vd[g % 2]
            esl = E_sb[:, y0:y0 + 4, :]
            e_bc = bass.AP(tensor=esl.tensor, offset=esl.offset,
                           ap=[list(esl.ap[0]), [NH, 4], [1, NH], [0, HD]])
            nc.vector.tensor_tensor(
                U_T[:, y0:y0 + 4, :, 0:HD],
                v[:, :, :].rearrange("p y (h g) -> p y h g", h=NH),
                e_bc, AL.mult)
            nc.vector.tensor_copy(U_T[:, y0:y0 + 4, :, HD], esl)

        # ---------- phase B: taps + divide (spread (chunk, head) units) ----------
        # C-groups of a completed chunk are interleaved between units so the
        # phase-C work spreads evenly across the tap work
        pending_c = []
        unit_q = []
        unit_n = 0

        def tap_unit(ch, h):
            nonlocal unit_n
            y0, rows = CHUNKS[ch]
            np_ps = npt[unit_n % 2]
            unit_n += 1
            for i in range(KS):
                nc.tensor.matmul(np_ps[:, 0:rows, :],
                                 band_t[:, h * KS + i, :],
                                 U_T[:, y0 + i:y0 + i + rows, h, :],
                                 start=(i == 0), stop=(i == KS - 1))
            mp = np_ps[:, :, :]
            nh_half = (rows + 7) // 8   # 1 divide for 7 rows, 2 for 14
            step = rows // nh_half
            for half in range(nh_half):
                ys = half * step
                zbc = bass.AP(tensor=mp.tensor,
                              offset=mp.offset + ys * (HD + 1) + HD,
                              ap=[list(mp.ap[0]), [HD + 1, step], [0, HD]])
                nc.gpsimd.tensor_tensor(
                    attn_sb[:, y0 + ys:y0 + ys + step,
                            h * HD:(h + 1) * HD],
                    np_ps[:, ys:ys + step, 0:HD], zbc, AL.divide)
            if h == NH - 1:
                pending_c.extend(AFTER_B[ch])

        def pop_units(k):
            while k > 0 and unit_q:
                ch, h = unit_q.pop(0)
                tap_unit(ch, h)
                if pending_c:
                    run_c(pending_c.pop(0))
                k -= 1

        # ---------- phase C: transpose -> proj -> out (per 4-row group) ----------
        c_state = {"t": None, "nat": None, "out_sb": None}

        def c_front(t):
            y0 = 4 * t
            s0 = (t % 2) * 4
            for r in range(4):
                nc.tensor.transpose(tp[:, s0 + r, :], attn_sb[:, y0 + r, :],
                                    ident_t[:])
            attn_nat = work.tile([C, TN], BF16, tag="attn_nat")
            nc.scalar.activation(attn_nat[:, :].rearrange("p (a b) -> p a b", a=4),
                                 tp[:, s0:s0 + 4, :], AF.Copy)
            return attn_nat

        def c_back(t, attn_nat, out_sb):
            nc.tensor.matmul(pod[:, 0:TN], projw_t[:], attn_nat[:],
                             start=True, stop=True)
            s = (t % 4) * TN
            nc.vector.tensor_scalar(out_sb[:, s:s + TN], pod[:, 0:TN],
                                    pbias_t[:, 0:1], None, AL.add)
            if t // 4 == 6:     # last quarter: store per group (short tail)
                nc.sync.dma_start(
                    out=out_d.ap()[:, 4 * t * W:4 * (t + 1) * W],
                    in_=out_sb[:, s:s + TN])
            elif t % 4 == 3:
                q = t // 4
                nc.sync.dma_start(
                    out=out_d.ap()[:, q * 4 * TN:(q + 1) * 4 * TN],
                    in_=out_sb)

        def run_c(t):
            # pipelined: emit proj/out for the previous group, then
            # transposes for this one
            if c_state["t"] is not None:
                c_back(c_state["t"], c_state["nat"], c_state["out_sb"])
            if t is None:
                c_state["t"] = None
                return
            if t % 4 == 0:
                c_state["out_sb"] = outp.tile([C, 4 * TN], BF16, tag="out_sb",
                                              name="out_sb")
            c_state["nat"] = c_front(t)
            c_state["t"] = t

        # ---------- emission schedule (software-pipelined, lag 1) ----------
        qk_prev = None
        for g in range(NG):
            qk_cur = a_front(g)
            if g >= 1:
                a_back(g - 1, qk_prev)
            qk_prev = qk_cur
            if g == 2:
                # y halo: rows 112..117 = rows 0..5 (incl. Z channel)
                nc.vector.tensor_copy(U_T[:, H:YP, :, :], U_T[:, 0:KS - 1, :, :])
            for ch in GATES.get(g, []):
                unit_q.extend((ch, h) for h in range(NH))
            pop_units(2)
        a_back(NG - 1, qk_prev)
        for ch in GATES.get(NG + 1, []):
            unit_q.extend((ch, h) for h in range(NH))
        pop_units(len(unit_q))
        while pending_c:
            run_c(pending_c.pop(0))
        run_c(None)

    nc.compile()
    return nc


def prep_inputs(x_b, qkv_w, qkv_b, rpb, proj_w, proj_b, zero_qkv_bias=True):
    """Host-side preprocessing of one batch element + shared weights."""
    qkv_w = np.asarray(qkv_w, dtype=np.float32)
    qkv_b = np.asarray(qkv_b, dtype=np.float32)
    rpb = np.asarray(rpb, dtype=np.float32).reshape(NH, KS, KS)
    proj_w = np.asarray(proj_w, dtype=np.float32)
    proj_b = np.asarray(proj_b, dtype=np.float32)

    wq = (SCALE * qkv_w[0:C]).T.astype(bfloat16)          # [a, c_out]
    wk = qkv_w[C:2 * C].T.astype(bfloat16)
    wvT = qkv_w[2 * C:3 * C].T.astype(bfloat16)
    bv = qkv_b[2 * C:3 * C]
    hm4 = np.zeros((C, NH), np.float32)
    for h in range(NH):
        hm4[h * HD:(h + 1) * HD, h] = 1.0
    hm4 = hm4.astype(bfloat16)
    R = np.exp(rpb)                                        # [NH, KS, KS]
    xs = np.arange(W)[:, None]
    xd = np.arange(W)[None, :]
    jm = (xs - xd) % W
    mask = jm < KS
    jc = np.minimum(jm, KS - 1)
    band = np.zeros((W, NH, KS, W), np.float32)
    for h in range(NH):
        for i in range(KS):
            band[:, h, i, :] = np.where(mask, R[h, i][jc], 0.0)
    band = band.reshape(W, NH * KS * W).astype(bfloat16)
    projw = proj_w.T.astype(bfloat16)
    pbias = (proj_w @ bv + proj_b).reshape(C, 1).astype(np.float32)
    ident = np.eye(W, dtype=bfloat16)
    m = {"x": x_b.astype(bfloat16), "wq": wq, "wk": wk, "wvT": wvT,
         "hm4": hm4, "band": band, "projw": projw, "ident": ident,
         "pbias": pbias}
    if not zero_qkv_bias:
        m["bq"] = (SCALE * qkv_b[0:C]).reshape(C, 1).astype(np.float32)
        m["bk"] = qkv_b[C:2 * C].reshape(C, 1).astype(np.float32)
    return m


_NC = None
_NC_ZB = None


def kernel(x, qkv_w, qkv_b, rpb, proj_w, proj_b):
    global _NC, _NC_ZB
    qkv_b = np.asarray(qkv_b, dtype=np.float32)
    zb = bool(np.all(qkv_b[0:2 * C] == 0.0))
    if _NC is None or _NC_ZB != zb:
        _NC = build_nc(zero_qkv_bias=zb)
        _NC_ZB = zb
    x = np.ascontiguousarray(np.asarray(x, dtype=np.float32))
    shared = prep_inputs(np.zeros((C, HW), np.float32),
                         qkv_w, qkv_b, rpb, proj_w, proj_b, zero_qkv_bias=zb)
    in_maps = []
    for b in range(B):
        m = dict(shared)
        m["x"] = x[b].reshape(C, HW).astype(bfloat16)
        in_maps.append(m)
    res = run_bass_kernel_spmd(_NC, in_maps, list(range(B)), trace=False)
    return np.stack([np.asarray(res.results[b]["out"], dtype=np.float32)
                     .reshape(C, H, W) for b in range(B)])


# revision 19
# speedup vs baseline: 1.1389x; 1.0622x over previous
"""NeighborhoodAttention2D Trainium2 kernel (8-core data parallel over batch).

Math (matches reference.py):
  dot(h,p)  = sum_{c in head h} s*(q_c(p)+bq_c)*(k_c(p)+bk_c),  s = hd^-0.5
  logit_ij(p) = dot(p + (i,j)) + rpb[h,i,j]      (circular shifts)
  attn      = softmax over the 49 (i,j)
  out_attn(c,p) = sum_ij attn_ij(p) * (v_c + bv_c)(p + (i,j))
Max-free softmax: E = exp(dot); R = exp(rpb);
  num = sum_ij R_ij*(E*v)_shift;  Z = sum_ij R_ij*E_shift;  out = num/Z + bv
(bv folded into the proj bias: proj_w @ bv + proj_b.)

Layout strategy: the 49-tap conv runs in a spatially-transposed layout
(partitions = x) as banded-circulant bf16 matmuls on TensorE:
  num_T[xd, y, c] = sum_i sum_xs band[h,i][xs,xd] * U_T[xs, y+i, c]
where band[h,i][xs,xd] = R[h,i,(xs-xd) mod 112] masks a 7-wide circulant.
U_T = E*(v+bv) is produced directly in transposed form by using the x tile
as the matmul lhsT (out partitions = positions). The x-wrap is inside the
band matrices; the y-wrap uses 6 halo rows. Z rides along as a 33rd
channel per head and the num/Z division happens on the Pool engine with a
stride-0-broadcast psum divisor. All weight preprocessing (transposes,
scale folding, exp(rpb), band construction, bf16 casts) happens on the
host in numpy.
"""
import sys
import contextlib
import numpy as np

sys.path.insert(0, '/opt/trn_rl_repo')

import concourse.bass as bass
import concourse.bacc as bacc
import concourse.mybir as mybir
from concourse import tile
from concourse.bass_utils import run_bass_kernel_spmd
from ml_dtypes import bfloat16

# ---- problem constants ----
B, C, H, W = 8, 128, 112, 112
NH, HD, KS = 4, 32, 7
HW = H * W                        # 12544
YP = H + KS - 1                   # 118 (y-padded)
SCALE = HD ** (-0.5)
NG = H // 4                       # 28 4-row groups
TN = 4 * W                        # 448 positions per group
NCH = 8                           # tap chunks (14 rows each)
CHR = H // NCH                    # 14 rows per chunk

F32 = mybir.dt.float32
BF16 = mybir.dt.bfloat16
AL = mybir.AluOpType
AF = mybir.ActivationFunctionType

# tap chunks (y0, rows): small early chunks start tap work sooner
CHUNKS = [(14 * i, 14) for i in range(8)]

def _chunk_of_row(r):
    for ci, (y0, rows) in enumerate(CHUNKS):
        if y0 <= r < y0 + rows:
            return ci
    raise AssertionError

# iteration at which chunk ci's tap units may be emitted: the taps read
# input rows y0..y0+rows+5, which exist once a_back(g-1) covered them
GATES = {}
for ci, (y0, rows) in enumerate(CHUNKS):
    need = y0 + rows + 5            # top input row (halo-free chunks)
    g = -(-(need + 1) // 4) + 2     # extra iterations of slack
    if need >= H:
        g = NG + 1                  # post-loop (needs halo)
    GATES.setdefault(min(g, NG + 1), []).append(ci)
AFTER_B = {ci: [t for t in range(NG) if _chunk_of_row(4 * t + 3) == ci]
           for ci in range(len(CHUNKS))}


def build_nc(zero_qkv_bias=True):
    nc = bacc.Bacc(target_bir_lowering=False)

    x_d = nc.dram_tensor("x", [C, HW], BF16, kind="ExternalInput")
    wq_d = nc.dram_tensor("wq", [C, C], BF16, kind="ExternalInput")
    wk_d = nc.dram_tensor("wk", [C, C], BF16, kind="ExternalInput")
    wvT_d = nc.dram_tensor("wvT", [C, C], BF16, kind="ExternalInput")
    hm4_d = nc.dram_tensor("hm4", [C, NH], BF16, kind="ExternalInput")
    band_d = nc.dram_tensor("band", [W, NH * KS * W], BF16, kind="ExternalInput")
    projw_d = nc.dram_tensor("projw", [C, C], BF16, kind="ExternalInput")
    ident_d = nc.dram_tensor("ident", [W, W], BF16, kind="ExternalInput")
    pbias_d = nc.dram_tensor("pbias", [C, 1], F32, kind="ExternalInput")
    if not zero_qkv_bias:
        bq_d = nc.dram_tensor("bq", [C, 1], F32, kind="ExternalInput")
        bk_d = nc.dram_tensor("bk", [C, 1], F32, kind="ExternalInput")
    out_d = nc.dram_tensor("out", [C, HW], BF16, kind="ExternalOutput")

    with tile.TileContext(nc) as tc, contextlib.ExitStack() as ctx:
        sing = ctx.enter_context(tc.tile_pool(name="sing", bufs=1))
        work = ctx.enter_context(tc.tile_pool(name="work", bufs=2))
        outp = ctx.enter_context(tc.tile_pool(name="outp", bufs=2))
        psum = ctx.enter_context(tc.tile_pool(name="psum", bufs=1, space="PSUM"))

        # ---- PSUM: 8 banks, statically placed, subtile deps ----
        qp = psum.tile([C, TN], F32, tag="qp")
        kp = psum.tile([C, TN], F32, tag="kp")
        vd = [psum.tile([W, 4, C], F32, tag=f"vd{i}", name=f"vd{i}")
              for i in range(2)]
        npt = [psum.tile([W, CHR, HD + 1], F32, tag=f"np{i}", name=f"np{i}")
               for i in range(2)]
        tp = psum.tile([C, 8, W], BF16, tag="tp")
        pod = psum.tile([C, TN + 8 * NH], F32, tag="pod")  # proj | 2x dot regions

        # ---- weights (scalar queue) and x (sync queue), first-needed first ----
        wq_t = sing.tile([C, C], BF16, tag="wq")
        wk_t = sing.tile([C, C], BF16, tag="wk")
        wvT_t = sing.tile([C, C], BF16, tag="wvT")
        hm4_t = sing.tile([C, NH], BF16, tag="hm4")
        band_t = sing.tile([W, NH * KS, W], BF16, tag="band")
        projw_t = sing.tile([C, C], BF16, tag="projw")
        ident_t = sing.tile([W, W], BF16, tag="ident")
        pbias_t = sing.tile([C, 1], F32, tag="pbias")
        x_bf = sing.tile([C, HW], BF16, tag="x_bf")

        nc.sync.dma_start(out=wq_t, in_=wq_d.ap())
        nc.sync.dma_start(out=wk_t, in_=wk_d.ap())
        # x chunks: rows [0:8), [8:16), then 16-row chunks
        xrow = [0, 4, 8, 16, 32, 48, 64, 80, 96, 112]
        for d in range(len(xrow) - 1):
            r0, r1 = xrow[d], xrow[d + 1]
            nc.scalar.dma_start(out=x_bf[:, r0 * W:r1 * W],
                                in_=x_d.ap()[:, r0 * W:r1 * W])
        nc.sync.dma_start(out=wvT_t, in_=wvT_d.ap())
        nc.sync.dma_start(out=hm4_t, in_=hm4_d.ap())
        if not zero_qkv_bias:
            bq_t = sing.tile([C, 1], F32, tag="bq")
            bk_t = sing.tile([C, 1], F32, tag="bk")
            nc.sync.dma_start(out=bq_t, in_=bq_d.ap())
            nc.sync.dma_start(out=bk_t, in_=bk_d.ap())
        nc.sync.dma_start(out=band_t[:, :, :].rearrange("p a b -> p (a b)"),
                            in_=band_d.ap())
        nc.sync.dma_start(out=projw_t, in_=projw_d.ap())
        nc.sync.dma_start(out=ident_t, in_=ident_d.ap())
        nc.sync.dma_start(out=pbias_t, in_=pbias_d.ap())

        U_T = sing.tile([W, YP, NH, HD + 1], BF16, tag="U_T")
        E_sb = sing.tile([W, H, NH], F32, tag="E_sb")
        attn_sb = sing.tile([W, H, C], BF16, tag="attn_sb")

        dotv = [pod[0:W, TN + 16 * i:TN + 16 * (i + 1)]
            .rearrange("p (a b) -> p a b", a=4) for i in range(2)]

        # ---------- phase A (per 4-row group) ----------
        def a_front(g):
            y0 = 4 * g
            nc.tensor.matmul(qp[:], wq_t[:], x_bf[:, y0 * W:(y0 + 4) * W],
                             start=True, stop=True)
            nc.tensor.matmul(kp[:], wk_t[:], x_bf[:, y0 * W:(y0 + 4) * W],
                             start=True, stop=True)
            v = vd[g % 2]
            for r in range(4):
                nc.tensor.matmul(v[:, r, :],
                                 x_bf[:, (y0 + r) * W:(y0 + r + 1) * W],
                                 wvT_t[:], start=True, stop=True)
            qk_bf = work.tile([C, TN], BF16, tag="qk_bf")
            if zero_qkv_bias:
                nc.vector.tensor_tensor(qk_bf[:], qp[:], kp[:], AL.mult)
            else:
                k_sb = work.tile([C, TN], BF16, tag="k_sb")
                nc.scalar.activation(k_sb[:], kp[:], AF.Identity,
                                     bias=bk_t[:, 0:1])
                nc.vector.scalar_tensor_tensor(qk_bf[:], qp[:], bq_t[:, 0:1],
                                               k_sb[:], AL.add, AL.mult)
            return qk_bf

        def a_back(g, qk_bf):
            y0 = 4 * g
            dv = dotv[g % 2]
            for r in range(4):
                nc.tensor.matmul(dv[:, r, :],
                                 qk_bf[:, r * W:(r + 1) * W], hm4_t[:],
                                 start=True, stop=True)
            nc.scalar.activation(E_sb[:, y0:y0 + 4, :], dv[:, :, :], AF.Exp)
            v = vd[g % 2]
            esl = E_sb[:, y0:y0 + 4, :]
            e_bc = bass.AP(tensor=esl.tensor, offset=esl.offset,
                           ap=[list(esl.ap[0]), [NH, 4], [1, NH], [0, HD]])
            nc.vector.tensor_tensor(
                U_T[:, y0:y0 + 4, :, 0:HD],
                v[:, :, :].rearrange("p y (h g) -> p y h g", h=NH),
                e_bc, AL.mult)
            nc.vector.tensor_copy(U_T[:, y0:y0 + 4, :, HD], esl)

        # ---------- phase B: taps + divide (spread (chunk, head) units) ----------
        # C-groups of a completed chunk are interleaved between units so the
        # phase-C work spreads evenly across the tap work
        pending_c = []
        unit_q = []
        unit_n = 0

        def tap_unit(ch, h):
            nonlocal unit_n
            y0, rows = CHUNKS[ch]
            np_ps = npt[unit_n % 2]
            unit_n += 1
            for i in range(KS):
                nc.tensor.matmul(np_ps[:, 0:rows, :],
                                 band_t[:, h * KS + i, :],
                                 U_T[:, y0 + i:y0 + i + rows, h, :],
                                 start=(i == 0), stop=(i == KS - 1))
            mp = np_ps[:, :, :]
            nh_half = (rows + 7) // 8   # 1 divide for 7 rows, 2 for 14
            step = rows // nh_half
            for half in range(nh_half):
                ys = half * step
                zbc = bass.AP(tensor=mp.tensor,
                              offset=mp.offset + ys * (HD + 1) + HD,
                              ap=[list(mp.ap[0]), [HD + 1, step], [0, HD]])
                nc.gpsimd.tensor_tensor(
                    attn_sb[:, y0 + ys:y0 + ys + step,
                            h * HD:(h + 1) * HD],
                    np_ps[:, ys:ys + step, 0:HD], zbc, AL.divide)
            if h == NH - 1:
                pending_c.extend(AFTER_B[ch])

        def pop_units(k):
            while k > 0 and unit_q:
                ch, h = unit_q.pop(0)
                tap_unit(ch, h)
                if pending_c:
                    run_c(pending_c.pop(0))
                k -= 1

        # ---------- phase C: transpose -> proj -> out (per 4-row group) ----------
        c_state = {"t": None, "nat": None, "out_sb": None}

        def c_front(t):
            y0 = 4 * t
            s0 = (t % 2) * 4
            for r in range(4):
                nc.tensor.transpose(tp[:, s0 + r, :], attn_sb[:, y0 + r, :],
                                    ident_t[:])
            attn_nat = work.tile([C, TN], BF16, tag="attn_nat")
            nc.scalar.activation(attn_nat[:, :].rearrange("p (a b) -> p a b", a=4),
                                 tp[:, s0:s0 + 4, :], AF.Copy)
            return attn_nat

        def c_back(t, attn_nat, out_sb):
            nc.tensor.matmul(pod[:, 0:TN], projw_t[:], attn_nat[:],
                             start=True, stop=True)
            s = (t % 4) * TN
            nc.vector.tensor_scalar(out_sb[:, s:s + TN], pod[:, 0:TN],
                                    pbias_t[:, 0:1], None, AL.add)
            if t // 4 == 6:     # last quarter: store per group (short tail)
                nc.sync.dma_start(
                    out=out_d.ap()[:, 4 * t * W:4 * (t + 1) * W],
                    in_=out_sb[:, s:s + TN])
            elif t % 4 == 3:
                q = t // 4
                nc.sync.dma_start(
                    out=out_d.ap()[:, q * 4 * TN:(q + 1) * 4 * TN],
                    in_=out_sb)

        def run_c(t):
            # pipelined: emit proj/out for the previous group, then
            # transposes for this one
            if c_state["t"] is not None:
                c_back(c_state["t"], c_state["nat"], c_state["out_sb"])
            if t is None:
                c_state["t"] = None
                return
            if t % 4 == 0:
                c_state["out_sb"] = outp.tile([C, 4 * TN], BF16, tag="out_sb",
                                              name="out_sb")
            c_state["nat"] = c_front(t)
            c_state["t"] = t

        # ---------- emission schedule (software-pipelined, lag 1) ----------
        qk_prev = None
        for g in range(NG):
            qk_cur = a_front(g)
            if g >= 1:
                a_back(g - 1, qk_prev)
            qk_prev = qk_cur
            if g == 2:
                # y halo: rows 112..117 = rows 0..5 (incl. Z channel)
                nc.vector.tensor_copy(U_T[:, H:YP, :, :], U_T[:, 0:KS - 1, :, :])
            for ch in GATES.get(g, []):
                unit_q.extend((ch, h) for h in range(NH))
            pop_units(2)
        a_back(NG - 1, qk_prev)
        for ch in GATES.get(NG + 1, []):
            unit_q.extend((ch, h) for h in range(NH))
        pop_units(len(unit_q))
        while pending_c:
            run_c(pending_c.pop(0))
        run_c(None)

    nc.compile()
    return nc


def prep_inputs(x_b, qkv_w, qkv_b, rpb, proj_w, proj_b, zero_qkv_bias=True):
    """Host-side preprocessing of one batch element + shared weights."""
    qkv_w = np.asarray(qkv_w, dtype=np.float32)
    qkv_b = np.asarray(qkv_b, dtype=np.float32)
    rpb = np.asarray(rpb, dtype=np.float32).reshape(NH, KS, KS)
    proj_w = np.asarray(proj_w, dtype=np.float32)
    proj_b = np.asarray(proj_b, dtype=np.float32)

    wq = (SCALE * qkv_w[0:C]).T.astype(bfloat16)          # [a, c_out]
    wk = qkv_w[C:2 * C].T.astype(bfloat16)
    wvT = qkv_w[2 * C:3 * C].T.astype(bfloat16)
    bv = qkv_b[2 * C:3 * C]
    hm4 = np.zeros((C, NH), np.float32)
    for h in range(NH):
        hm4[h * HD:(h + 1) * HD, h] = 1.0
    hm4 = hm4.astype(bfloat16)
    R = np.exp(rpb)                                        # [NH, KS, KS]
    xs = np.arange(W)[:, None]
    xd = np.arange(W)[None, :]
    jm = (xs - xd) % W
    mask = jm < KS
    jc = np.minimum(jm, KS - 1)
    band = np.zeros((W, NH, KS, W), np.float32)
    for h in range(NH):
        for i in range(KS):
            band[:, h, i, :] = np.where(mask, R[h, i][jc], 0.0)
    band = band.reshape(W, NH * KS * W).astype(bfloat16)
    projw = proj_w.T.astype(bfloat16)
    pbias = (proj_w @ bv + proj_b).reshape(C, 1).astype(np.float32)
    ident = np.eye(W, dtype=bfloat16)
    m = {"x": x_b.astype(bfloat16), "wq": wq, "wk": wk, "wvT": wvT,
         "hm4": hm4, "band": band, "projw": projw, "ident": ident,
         "pbias": pbias}
    if not zero_qkv_bias:
        m["bq"] = (SCALE * qkv_b[0:C]).reshape(C, 1).astype(np.float32)
        m["bk"] = qkv_b[C:2 * C].reshape(C, 1).astype(np.float32)
    return m


_NC = None
_NC_ZB = None


def kernel(x, qkv_w, qkv_b, rpb, proj_w, proj_b):
    global _NC, _NC_ZB
    qkv_b = np.asarray(qkv_b, dtype=np.float32)
    zb = bool(np.all(qkv_b[0:2 * C] == 0.0))
    if _NC is None or _NC_ZB != zb:
        _NC = build_nc(zero_qkv_bias=zb)
        _NC_ZB = zb
    x = np.ascontiguousarray(np.asarray(x, dtype=np.float32))
    shared = prep_inputs(np.zeros((C, HW), np.float32),
                         qkv_w, qkv_b, rpb, proj_w, proj_b, zero_qkv_bias=zb)
    in_maps = []
    for b in range(B):
        m = dict(shared)
        m["x"] = x[b].reshape(C, HW).astype(bfloat16)
        in_maps.append(m)
    res = run_bass_kernel_spmd(_NC, in_maps, list(range(B)), trace=False)
    return np.stack([np.asarray(res.results[b]["out"], dtype=np.float32)
                     .reshape(C, H, W) for b in range(B)])
